# revision 8
# baseline (speedup 1.0000x reference)
"""Trainium2 Bass kernel: causal depthwise Conv1d (K=4) + SiLU.

Reference computation (B=4, S=4096, D=2048):
    y[b, s, d] = silu( sum_k w[d, 0, k] * x[b, s-3+k, d] )   (zero-padded left)

Strategy (v2):
  * Host: transpose x to channel-major (D, B, S), left-pad each row with
    4 zeros (row length 4100), cast to bf16, shard D across the 8
    NeuronCores (256 channels each).  Depthwise conv is channel-independent
    -> no inter-core communication.
  * Core: 8 tiles of [128, 4100].  Measured engine rates (NTFF):
      PE:  one 512-col matmul per ~218ns (LDW hidden)  -> 1.71 ns/col/tile
      DVE: 4 tensor_scalar @0.27ns/col + 3 adds @0.53ns/col -> 2.67 ns/col
      ACT: silu 0.92 ns/col + ~170ns/inst
      HBM: ~412 GB/s aggregate; 17.05 MB in+out -> 41.4us floor
    Tiles 0,2,3,5,7 on PE (diag-stationary matmuls, 4-tap accumulation in
    PSUM, ACT silu drains PSUM->bf16), tiles 1,4,6 on DVE.  Units are
    emitted in modeled completion order so the strict-FIFO ACT queue never
    head-of-line blocks.
  * Outputs: per-2048-chunk DMAs; early ones on gpsimd (SWDGE), late ones
    alternate sync/scalar (HWDGE, short completion receipt) so the kernel
    tail closes fast.
  * Host: gather, transpose back, cast to f32.
"""

import os
import sys

sys.path.insert(0, "/opt/trn_rl_repo")

import numpy as np
import ml_dtypes

N_CORES = 8
B, S, D = 4, 4096, 2048
K = 4
PAD = 4
ROW = S + PAD  # 4100
D_LOCAL = D // N_CORES  # 256
G = D_LOCAL // 128  # 2 partition groups per core

MM_N = 512  # PSUM bank limit (512 f32)
WARMUP_MMS = int(os.environ.get("KERNEL_WARMUP", "6"))
PE_CHUNK = int(os.environ.get("KERNEL_PE_CHUNK", "2048"))
# columns of tile 7 computed on DVE (rest on PE); 0 = tile 7 fully on PE
SPLIT7_DVE = int(os.environ.get("KERNEL_SPLIT7_DVE", "0"))
# modeled-completion threshold (ns) after which outputs use the sync HWDGE
# queue instead of gpsimd SWDGE
HW_OUT_NS = float(os.environ.get("KERNEL_HW_OUT_NS", "32000"))

_CACHE = {}

# ---- measured cost model (ns) for emission ordering ----------------------
PE_START = 8300.0
DVE_START = 9500.0
PE_NS_PER_COL = 1.71  # 4 taps, 218ns per 512-col matmul
DVE_TS_NS = lambda w: w * 0.27 + 170.0
DVE_TT_NS = lambda w: w * 0.53 + 100.0
POOL_TT_NS = lambda w: w * 1.2 + 400.0
ACT_NS = lambda w: w * 0.92 + 170.0


def _dve_chain_ns(w, pool_add):
    # 4 muls + 2 pair-adds on DVE; the final add is on DVE unless offloaded
    # to gpsimd (pool_add)
    return 4 * DVE_TS_NS(w) + (2 if pool_add else 3) * DVE_TT_NS(w)


def _build():
    import concourse.tile as tile
    from concourse import bacc, mybir

    nc = bacc.Bacc("TRN2", debug=False, enable_asserts=False, num_devices=N_CORES)
    bf16 = mybir.dt.bfloat16
    f32 = mybir.dt.float32

    x_ap = nc.dram_tensor("x", [G, 128, B, ROW], bf16, kind="ExternalInput").ap()
    wd_ap = nc.dram_tensor("wd", [128, G * K * 128], bf16, kind="ExternalInput").ap()
    w_ap = nc.dram_tensor("w", [128, G * K], f32, kind="ExternalInput").ap()
    out_ap = nc.dram_tensor("out", [G, 128, B, S], bf16, kind="ExternalOutput").ap()

    DVE_TILES = (1, 4, 6)
    NT = G * B  # 8

    with tile.TileContext(nc) as tc:
        with (
            tc.tile_pool(name="wp", bufs=1) as wp,
            tc.tile_pool(name="xp", bufs=8) as xp,
            tc.tile_pool(name="tp", bufs=2) as tp,
            tc.tile_pool(name="cp", bufs=2) as cp,
            tc.tile_pool(name="ps", bufs=2, space="PSUM") as ps,
            tc.tile_pool(name="yp", bufs=4) as yp,
        ):
            wd = wp.tile([128, G * K * 128], bf16, tag="wd")
            wt = wp.tile([128, G * K], f32, tag="wt")

            def wdiag(g, k):
                c0 = (g * K + k) * 128
                return wd[:, c0 : c0 + 128]

            def wcol(g, k):
                return wt[:, g * K + k : g * K + k + 1]

            # HAM warmup: dummy matmuls on a zeroed stationary keep the PE
            # p-state ramping through the ~3us window so real chunks run at
            # full clock.  Gated only on a gpsimd memset; result never read.
            if WARMUP_MMS:
                zt = wp.tile([128, MM_N], bf16, tag="zt")
                nc.gpsimd.memset(zt[:], 0)
                warm = ps.tile([128, PE_CHUNK], f32, tag="acc")
                for _ in range(WARMUP_MMS):
                    nc.tensor.matmul(
                        warm[:, 0:MM_N], zt[:, 0:128], zt[:], start=True, stop=True
                    )

            # ---- input DMAs (sync queue, HWDGE) --------------------------
            # Tiles 0 (PE) and 1 (DVE) stream in 3 chunks for fast engine
            # ramp; the rest arrive in 2 halves.  Chunk bounds sit 4 cols
            # past each compute boundary (a chunk reads up to lo+W+3+1).
            # Tile 1's chunks come FIRST: the DVE pipeline is the longest
            # (chain + 2 silus), so it must start as early as possible.
            tile_bounds = {}
            for ti in range(NT):
                if ti in (0, 1):
                    tile_bounds[ti] = [0, 1028, 2052, ROW]
                else:
                    tile_bounds[ti] = [0, 2052, ROW]
            dma_order = [
                (1, 0), (0, 0), (0, 1), (1, 1), (0, 2), (1, 2),
                (2, 0), (2, 1), (3, 0), (4, 0), (3, 1), (4, 1),
                (5, 0), (5, 1), (6, 0), (6, 1), (7, 0), (7, 1),
            ]
            xts = [None] * NT
            for ti in range(NT):
                xt = xp.tile([128, ROW], bf16, tag="xt")
                xts[ti] = xt
            nc.sync.dma_start(out=wd[:], in_=wd_ap[:])
            nc.sync.dma_start(out=wt[:], in_=w_ap[:])
            in_done_ns = {ti: [] for ti in range(NT)}
            t_in = 7900.0
            IN_RATE = 0.35  # ~350 GB/s early input share (bytes/ns)
            for ti, ci in dma_order:
                g, b = divmod(ti, B)
                c0, c1 = tile_bounds[ti][ci], tile_bounds[ti][ci + 1]
                nc.sync.dma_start(out=xts[ti][:, c0:c1], in_=x_ap[g, :, b, c0:c1])
                t_in += (c1 - c0) * 128 * 2 / (IN_RATE * 1000.0)
                in_done_ns[ti].append((c1, t_in))

            def arrival(ti, col_hi):
                # modeled time the input covering [0, col_hi+PAD) has landed
                for c1, t in sorted(in_done_ns[ti]):
                    if c1 >= min(col_hi + PAD, ROW):
                        return t
                return max(t for _, t in in_done_ns[ti])

            # ---- build unit worklist with modeled completion times -------
            # kinds: "pe" (chunk: matmuls), "dvec" (vector chain; chains of
            # tiles 1/4 offload the final add to gpsimd as "padd"),
            # then per-2048 "silu" units (ACT) and output DMAs.
            POOL_ADD_TILES = {1, 4}
            units = []  # (ready_ns, kind, ti, lo, hi)
            t_pe = PE_START
            pe_tiles = [ti for ti in range(NT) if ti not in DVE_TILES]
            for ti in pe_tiles:
                if ti == 0:
                    chunks = [(0, 1024), (1024, 2048), (2048, 3072), (3072, S)]
                else:
                    lo0 = SPLIT7_DVE if ti == 7 else 0
                    chunks = []
                    c0 = lo0
                    while c0 < S:
                        chunks.append((c0, min(c0 + PE_CHUNK, S)))
                        c0 += PE_CHUNK
                for lo, hi in chunks:
                    t_pe = max(t_pe, arrival(ti, hi)) + (hi - lo) * PE_NS_PER_COL
                    units.append((t_pe, "pe", ti, lo, hi))

            t_dve = DVE_START
            dve_work = []
            for ti in DVE_TILES:
                if ti == 1:
                    dve_work += [(1, 0, 1024), (1, 1024, 2048), (1, 2048, S)]
                else:
                    dve_work.append((ti, 0, S))
            if SPLIT7_DVE:
                dve_work.append((7, 0, SPLIT7_DVE))
            for ti, lo, hi in dve_work:
                pool_add = ti in POOL_ADD_TILES
                t_dve = max(t_dve, arrival(ti, hi)) + _dve_chain_ns(hi - lo, pool_add)
                units.append((t_dve, "dvec", ti, lo, hi))
                if pool_add:
                    units.append(
                        (t_dve + POOL_TT_NS(hi - lo), "padd", ti, lo, hi)
                    )

            units.sort(key=lambda u: u[0])

            # silu+out units: 2048 granularity, after their producer (the
            # padd for pool-added chains, the chain/chunk itself otherwise)
            work = list(units)
            for t_done, kind, ti, lo, hi in units:
                if kind == "dvec" and ti in POOL_ADD_TILES:
                    continue  # silu hangs off the padd unit instead
                for c0 in range(lo, hi, 2048):
                    c1 = min(c0 + 2048, hi)
                    work.append((t_done + (c0 - lo) * 0.01 + 1.5, "silu", ti, c0, c1))
            work.sort(key=lambda u: u[0])

            cbufs = {}  # (ti, col) -> ("c"|"pp", bufs, chain lo)
            accs = {}  # (ti, lo) -> psum tile for PE results
            last_t = max(t for t, k, *_ in work if k == "silu")

            def emit_pe(ti, lo, hi):
                g, b = divmod(ti, B)
                xt = xts[ti]
                cw = hi - lo
                acc = ps.tile([128, cw], f32, tag="acc")
                accs[(ti, lo)] = acc
                for k in range(K):
                    for n0 in range(0, cw, MM_N):
                        xlo = lo + n0 + 1 + k
                        nw = min(MM_N, cw - n0)
                        nc.tensor.matmul(
                            acc[:, n0 : n0 + nw],
                            wdiag(g, k),
                            xt[:, xlo : xlo + nw],
                            start=(k == 0),
                            stop=(k == K - 1),
                        )

            def emit_dve_chain(ti, lo, hi):
                g, b = divmod(ti, B)
                xt = xts[ti]
                W = hi - lo
                t0 = tp.tile([128, W], bf16, tag="t0")
                nc.vector.tensor_scalar_mul(t0[:], xt[:, lo + 1 : lo + 1 + W], wcol(g, 0))
                t1 = tp.tile([128, W], bf16, tag="t1")
                nc.vector.tensor_scalar_mul(t1[:], xt[:, lo + 2 : lo + 2 + W], wcol(g, 1))
                p0 = cp.tile([128, W], bf16, tag="p0")
                nc.vector.tensor_add(p0[:], t0[:], t1[:])
                t2 = tp.tile([128, W], bf16, tag="t0")
                nc.vector.tensor_scalar_mul(t2[:], xt[:, lo + 3 : lo + 3 + W], wcol(g, 2))
                t3 = tp.tile([128, W], bf16, tag="t1")
                nc.vector.tensor_scalar_mul(t3[:], xt[:, lo + 4 : lo + 4 + W], wcol(g, 3))
                p1 = cp.tile([128, W], bf16, tag="p1")
                nc.vector.tensor_add(p1[:], t2[:], t3[:])
                if ti in POOL_ADD_TILES:
                    # final add happens later on gpsimd (padd unit)
                    for c0 in range(lo, hi, 2048):
                        cbufs[(ti, c0)] = ("pp", (p0, p1), lo)
                else:
                    c = cp.tile([128, W], bf16, tag="c")
                    nc.vector.tensor_add(c[:], p0[:], p1[:])
                    for c0 in range(lo, hi, 2048):
                        cbufs[(ti, c0)] = ("c", c, lo)

            def emit_padd(ti, lo, hi):
                W = hi - lo
                kind, (p0, p1), chain_lo = cbufs[(ti, lo)]
                c = cp.tile([128, W], bf16, tag="c")
                nc.gpsimd.tensor_add(c[:], p0[:], p1[:])
                for c0 in range(lo, hi, 2048):
                    cbufs[(ti, c0)] = ("c", c, lo)

            def emit_silu_out(t_done, ti, lo, hi):
                g, b = divmod(ti, B)
                W = hi - lo
                is_last = t_done >= last_t - 2500.0
                y = yp.tile([128, W], bf16, tag="y")
                sw = 1024 if is_last else W
                for s0 in range(0, W, sw):
                    scw = min(sw, W - s0)
                    if (ti, lo) in accs:
                        src, o = accs[(ti, lo)], s0
                    else:
                        kind, c, chain_lo = cbufs[(ti, lo)]
                        src, o = c, lo - chain_lo + s0
                    nc.scalar.activation(
                        out=y[:, s0 : s0 + scw],
                        in_=src[:, o : o + scw],
                        func=mybir.ActivationFunctionType.Silu,
                    )
                    if is_last:
                        nc.sync.dma_start(
                            out=out_ap[g, :, b, lo + s0 : lo + s0 + scw],
                            in_=y[:, s0 : s0 + scw],
                        )
                if not is_last:
                    if t_done < HW_OUT_NS:
                        nc.gpsimd.dma_start(out=out_ap[g, :, b, lo:hi], in_=y[:])
                    else:
                        nc.sync.dma_start(out=out_ap[g, :, b, lo:hi], in_=y[:])

            for t_done, kind, ti, lo, hi in work:
                if kind == "pe":
                    emit_pe(ti, lo, hi)
                elif kind == "dvec":
                    emit_dve_chain(ti, lo, hi)
                elif kind == "padd":
                    emit_padd(ti, lo, hi)
                else:
                    emit_silu_out(t_done, ti, lo, hi)

    nc.compile()
    return nc


def _get_nc():
    if "nc" not in _CACHE:
        _CACHE["nc"] = _build()
    return _CACHE["nc"]


def _make_in_maps(x, w):
    x = np.asarray(x, dtype=np.float32)
    w = np.asarray(w, dtype=np.float32)

    # (B, S, D) -> (D, B, S), bf16, left-pad rows with PAD zeros.
    x_t = np.ascontiguousarray(x.transpose(2, 0, 1)).astype(ml_dtypes.bfloat16)
    x_pad = np.zeros((D, B, ROW), dtype=ml_dtypes.bfloat16)
    x_pad[:, :, PAD:] = x_t
    w_flat = np.ascontiguousarray(w[:, 0, :])  # (D, K) f32

    in_maps = []
    for i in range(N_CORES):
        lo, hi = i * D_LOCAL, (i + 1) * D_LOCAL
        m = {"x": np.ascontiguousarray(x_pad[lo:hi].reshape(G, 128, B, ROW))}
        m["w"] = np.ascontiguousarray(
            w_flat[lo:hi].reshape(G, 128, K).transpose(1, 0, 2).reshape(128, G * K)
        )
        # diag stationaries, laid out [128, G*K*128] partition-first
        wd = np.zeros((G, K, 128, 128), dtype=ml_dtypes.bfloat16)
        wl = w_flat[lo:hi].reshape(G, 128, K).astype(ml_dtypes.bfloat16)
        idx = np.arange(128)
        for g in range(G):
            for k in range(K):
                wd[g, k, idx, idx] = wl[g, :, k]
        # (G,K,p,m) -> (p, G,K,m) -> [128, G*K*128]
        m["wd"] = np.ascontiguousarray(
            wd.transpose(2, 0, 1, 3).reshape(128, G * K * 128)
        )
        in_maps.append(m)
    return in_maps


def _assemble(results):
    parts = []
    for r in results:
        y = np.asarray(r["out"]).reshape(D_LOCAL, B, S)
        parts.append(y)
    y_full = np.concatenate(parts, axis=0)  # (D, B, S) bf16
    return np.ascontiguousarray(y_full.transpose(1, 2, 0)).astype(np.float32)


def kernel(x, w):
    from concourse.bass_utils import run_bass_kernel_spmd

    nc = _get_nc()
    in_maps = _make_in_maps(x, w)
    trace = bool(int(os.environ.get("KERNEL_TRACE", "0")))
    res = None
    err = None
    for attempt in range(3):
        try:
            res = run_bass_kernel_spmd(
                nc, in_maps, core_ids=list(range(N_CORES)),
                trace=trace and attempt == 0,
            )
            break
        except Exception as e:  # transient NRT device errors / missing trace hook
            err = e
            os.environ["BASS_NEVER_TRACE"] = "1"
            trace = False
    if res is None:
        raise err
    _CACHE["last_results"] = res
    return _assemble(res.results)
